# revision 1
# baseline (speedup 1.0000x reference)
"""Trainium2 Bass kernel for nn_BTGRule — j-sharded slotted design (v2).

Reference computation:
    L = span_rep @ Wl + bl            # [65, 65, 512]
    R = span_rep @ Wr + br            # [65, 65, 512]
    H = tanh(L[i, j] + R[j, k])       # over valid triples i < j < k
    scores[i, j, k] = H @ Wout + bout # [65, 65, 65, 2]

Sharding: split-point axis j is sharded across the 8 cores (each j's whole
(i, k) block lives on one core), so the L/R projections are computed once
total instead of once per core.  SPMD needs identical instruction streams,
so work is organized in 8 compile-time SLOTS of shape (A_s, W_s) =
(4(s+1), 64-4s).  A j-block of shape (j, 64-j) fits slot s with a = j
(normal orientation) or, transposed, a = 64-j.  Cores 0-3 take the normal
j in [1,32], cores 4-7 the transposed j in [33,63]; orientation is uniform
per core, so the only per-core difference is pure data: which spans go in
the "dense" (broadcast over a) vs "column" (broadcast over w) operand and
the order (Wr|Wl) vs (Wl|Wr) of the packed weights.

Per core per rep:
  PE:  dense/column projections per (slot-pair, hout) into one PSUM bank
       (+ bias via a 1-row ones matmul), then score matmuls vs Wout.
  DVE: one PSUM->SBUF f16 copy per (pair, hout); fused broadcast-add
       L+R per (slot, hout) via tensor_tensor with packed-pair APs
       (both operands 2-byte, last AP dim [1,2] -> 2x mode); 2/3 of the
       score copies.
  ACT: tanh per slot (slot-granular pipelining, emitted adds-first so DVE
       never starves the next tanh); 1/3 of the score copies.
  Host: packs spans/weights, scatters [2, 6528] per core to the dense
       [65,65,65,2] output and adds bout there.
  Timing builds unroll `inner` bodies per For_i iteration (the HW loop
       edge barriers engines; unrolling restores cross-rep overlap).
"""

import numpy as np

N1 = 65
HID = 512
HT = 4            # 128-row h tiles
OUT = 2
NCORES = 8

# slots: s -> (A, W); pair p couples slots (p, 7-p); physical order in H
SLOTS = [(4 * (s + 1), 64 - 4 * s) for s in range(8)]
PAIRS = [(0, 7), (1, 6), (2, 5), (3, 4)]
ORDER = [1, 6, 7, 5, 4, 3, 2, 0]        # slot processing order
SEQ = ORDER                              # H/out layout = processing order
# span DMA layout: pairs in first-use order
PAIR_USE = []
for _s in ORDER:
    _p = _s if _s <= 3 else 7 - _s
    if _p not in PAIR_USE:
        PAIR_USE.append(_p)
PAIR_POS = {p: i for i, p in enumerate(PAIR_USE)}
DW = 100          # dense cols per pair (W_sa + W_sb), same for all pairs
DC = 72           # doubled column cols per pair (2*(A_sa+A_sb))
SC = sum(a * w for a, w in SLOTS)        # 6528 H cols per core

# offsets in SEQ layout
_d_off, _c_off, _h_off, _s_off = {}, {}, {}, {}
_d, _c, _h = 0, 0, 0
for s in SEQ:
    A, W = SLOTS[s]
    _d_off[s], _c_off[s], _h_off[s], _s_off[s] = _d, _c, _h, _h
    _d += W
    _c += 2 * A
    _h += A * W
assert _d == 4 * DW and _c == 4 * DC and _h == SC

# pair offsets (contiguous in SEQ layout)
PAIR_D = {p: _d_off[sa] for p, (sa, sb) in enumerate(PAIRS)}
PAIR_C = {p: _c_off[sa] for p, (sa, sb) in enumerate(PAIRS)}
PAIR_H = {p: _h_off[sa] for p, (sa, sb) in enumerate(PAIRS)}


def _chunks(cols):
    n = -(-cols // 512)
    base = -(-cols // (2 * n)) * 2
    out = [base] * (n - 1) + [cols - base * (n - 1)]
    assert all(c % 2 == 0 and 0 < c <= 512 for c in out)
    return out


def jmap(core, s):
    """j hosted by (core, slot); None for the one dummy instance."""
    if core < 4:
        return 4 * s + 1 + core                 # normal, j in [1, 32]
    a = 4 * s + 1 + (core - 4)                  # transposed, a = 64 - j
    j = 64 - a
    return j if j >= 33 else None               # (core 7, slot 7) dummy


_COMPILED = None


def _build_program(reps=1, unroll=False, inner=1):
    import contextlib

    import concourse.bacc as bacc
    import concourse.mybir as mybir
    import concourse.tile as tile

    f32 = mybir.dt.float32
    f16 = mybir.dt.float16
    tanh = mybir.ActivationFunctionType.Tanh
    ident = mybir.ActivationFunctionType.Identity
    add = mybir.AluOpType.add

    nc = bacc.Bacc("TRN2", target_bir_lowering=False, debug=False,
                   num_devices=NCORES)

    wp_d = nc.declare_dram_parameter("wp", [128, 2 * HT * HID], f16,
                                     isOutput=False)
    spd_d = nc.declare_dram_parameter("spd", [128, HT * 4 * DW], f16,
                                      isOutput=False)
    spc_d = nc.declare_dram_parameter("spc", [128, HT * 4 * DC], f16,
                                      isOutput=False)
    misc_d = nc.declare_dram_parameter("misc", [128, HID + HT * OUT], f16,
                                       isOutput=False)
    out_d = nc.declare_dram_parameter("out", [OUT, SC], f32, isOutput=True)

    with tile.TileContext(nc) as tc:
        with (
            tc.tile_pool(name="const", bufs=1) as cpool,
            tc.tile_pool(name="stream", bufs=2) as spool,
            tc.tile_pool(name="ps_pr", bufs=4, space="PSUM") as ps_pr,
            tc.tile_pool(name="ps_sc", bufs=4, space="PSUM") as ps_sc,
            tc.For_i(0, reps // inner, 1,
                     hint_engines=(mybir.EngineType.PE,
                                   mybir.EngineType.DVE,
                                   mybir.EngineType.Activation,
                                   mybir.EngineType.SP))
            if reps > inner and not unroll else contextlib.nullcontext(),
        ):
          for _rep in range(reps if unroll else inner):
              misc_t = spool.tile([128, HID + HT * OUT], f16, tag="misc")
              blbr_t = misc_t[0:1, 0:HID]
              wout_t = misc_t[:, HID:HID + HT * OUT]
              wp_t = spool.tile([128, 2 * HT * HID], f16, tag="wp")
              spd_t = spool.tile([128, HT * 4 * DW], f16, tag="spd")
              spc_t = spool.tile([128, HT * 4 * DC], f16, tag="spc")
              # first-used pair's spans + hout-0 weights first
              nc.sync.dma_start(spd_t[:, 0:HT * DW], spd_d[:, 0:HT * DW])
              nc.sync.dma_start(spc_t[:, 0:HT * DC], spc_d[:, 0:HT * DC])
              nc.scalar.dma_start(wp_t[:, 0:1024], wp_d[:, 0:1024])
              for t in range(1, HT):
                  nc.scalar.dma_start(wp_t[:, t * 1024:(t + 1) * 1024],
                                      wp_d[:, t * 1024:(t + 1) * 1024])
              nc.sync.dma_start(spd_t[:, HT * DW:4 * HT * DW],
                                spd_d[:, HT * DW:4 * HT * DW])
              nc.sync.dma_start(spc_t[:, HT * DC:4 * HT * DC],
                                spc_d[:, HT * DC:4 * HT * DC])
              nc.scalar.dma_start(misc_t[:], misc_d[:])
              ones_t = cpool.tile([1, DC], f16, tag="ones")
              nc.vector.memset(ones_t[:], 1.0)

              # weight block: kind 0 = dense, 1 = column; hout t; hin hi
              def wblk(kind, t, hi):
                  c0 = t * 1024 + kind * HID + hi * 128
                  return wp_t[:, c0:c0 + 128]

              sbDC = spool.tile([128, 4 * HT * (DW + DC)], f16, tag="sbDC")
              H_t = cpool.tile([128, HT * SC], f16, tag="H")
              out_sb = spool.tile([OUT, SC], f32, tag="osb")

              def proj_pair(p, houts=range(HT)):
                  # projections for both slots of pair p
                  for t in houts:
                      ps = ps_pr.tile([128, DW + DC], f32, tag="pspr")
                      for hi in range(HT):
                          nc.tensor.matmul(
                              ps[:, 0:DW], wblk(0, t, hi),
                              spd_t[:, (PAIR_POS[p] * HT + hi) * DW:
                                    (PAIR_POS[p] * HT + hi) * DW + DW],
                              start=(hi == 0), stop=(hi == HT - 1))
                      for hi in range(HT):
                          nc.tensor.matmul(
                              ps[:, DW:DW + DC], wblk(1, t, hi),
                              spc_t[:, (PAIR_POS[p] * HT + hi) * DC:
                                    (PAIR_POS[p] * HT + hi) * DC + DC],
                              start=(hi == 0), stop=False)
                      nc.tensor.matmul(
                          ps[:, DW:DW + DC], blbr_t[0:1, t * 128:(t + 1) * 128],
                          ones_t[0:1, :], start=False, stop=True)
                      # one merged PSUM->SBUF f16 copy per (pair, hout)
                      g0 = (p * HT + t) * (DW + DC)
                      nc.vector.tensor_copy(sbDC[:, g0:g0 + DW + DC], ps[:])

              def adds_pair(p, only_slot=None):
                  for si, s in enumerate(PAIRS[p]):
                      if only_slot is not None and s != only_slot:
                          continue
                      A, W = SLOTS[s]
                      dd = 0 if si == 0 else SLOTS[PAIRS[p][0]][1]
                      cc = DW if si == 0 else DW + 2 * SLOTS[PAIRS[p][0]][0]
                      for t in range(HT):
                          h0 = HT * _h_off[s] + t * A * W
                          out_v = (H_t[:, h0:h0 + A * W]
                                   .rearrange("p (a w2 two) -> p a w2 two",
                                              a=A, two=2))
                          g0 = (p * HT + t) * (DW + DC)
                          in0 = (sbDC[:, g0 + dd:g0 + dd + W]
                                 .rearrange("p (w2 two) -> p w2 two", two=2)
                                 .unsqueeze(1)
                                 .broadcast_to([128, A, W // 2, 2]))
                          in1 = (sbDC[:, g0 + cc:g0 + cc + 2 * A]
                                 .rearrange("p (a two) -> p a two", two=2)
                                 .unsqueeze(2)
                                 .broadcast_to([128, A, W // 2, 2]))
                          nc.vector.tensor_tensor(out_v, in0, in1, op=add)

              def tanh_slot(s):
                  h0 = HT * _h_off[s]
                  n = HT * SLOTS[s][0] * SLOTS[s][1]
                  sec = H_t[:, h0:h0 + n]
                  nc.scalar.activation(sec, sec, tanh)

              def tanh_pair(p):
                  sa, sb = PAIRS[p]
                  h0 = HT * PAIR_H[p]
                  n = HT * (SLOTS[sa][0] * SLOTS[sa][1]
                            + SLOTS[sb][0] * SLOTS[sb][1])
                  sec = H_t[:, h0:h0 + n]
                  nc.scalar.activation(sec, sec, tanh)

              def scores_slot(s, outcnt=[0]):
                      A, W = SLOTS[s]
                      cols = A * W
                      c = 0
                      for ccw in _chunks(cols):
                          psc = ps_sc.tile([OUT, ccw], f32, tag="pssc")
                          for t in range(HT):
                              h0 = HT * _h_off[s] + t * cols
                              nc.tensor.matmul(
                                  psc[:], wout_t[:, OUT * t:OUT * (t + 1)],
                                  H_t[:, h0 + c:h0 + c + ccw],
                                  start=(t == 0), stop=(t == HT - 1))
                          dst = out_sb[:, _s_off[s] + c:_s_off[s] + c + ccw]
                          if outcnt[0] % 3 != 2:
                              nc.vector.tensor_copy(dst, psc[:])
                          else:
                              nc.scalar.activation(dst, psc[:], ident)
                          outcnt[0] += 1
                          c += ccw

              def scores_pair(p, outcnt=[0]):
                  for s in PAIRS[p]:
                      A, W = SLOTS[s]
                      cols = A * W
                      c = 0
                      for ccw in _chunks(cols):
                          psc = ps_sc.tile([OUT, ccw], f32, tag="pssc")
                          for t in range(HT):
                              h0 = HT * _h_off[s] + t * cols
                              nc.tensor.matmul(
                                  psc[:], wout_t[:, OUT * t:OUT * (t + 1)],
                                  H_t[:, h0 + c:h0 + c + ccw],
                                  start=(t == 0), stop=(t == HT - 1))
                          dst = out_sb[:, _s_off[s] + c:_s_off[s] + c + ccw]
                          if outcnt[0] % 3 != 2:
                              nc.vector.tensor_copy(dst, psc[:])
                          else:
                              nc.scalar.activation(dst, psc[:], ident)
                          outcnt[0] += 1
                          c += ccw

              # slot-granular pipeline over ORDER
              pair_of = {s2: p for p, pr in enumerate(PAIRS) for s2 in pr}
              seen = set()

              def need(s2):
                  p = pair_of[s2]
                  if p not in seen:
                      seen.add(p)
                      proj_pair(p)

              o = ORDER
              need(o[0])
              adds_pair(pair_of[o[0]], only_slot=o[0])
              need(o[1])
              tanh_slot(o[0])
              adds_pair(pair_of[o[1]], only_slot=o[1])
              for k in range(2, 8):
                  need(o[k])
                  tanh_slot(o[k - 1])
                  adds_pair(pair_of[o[k]], only_slot=o[k])
                  scores_slot(o[k - 2])
              tanh_slot(o[7])
              scores_slot(o[6])
              cut = _s_off[o[6]]      # slots o[0..5] are laid out before o[6]
              nc.sync.dma_start(out_d[:, 0:cut], out_sb[:, 0:cut])
              scores_slot(o[7])
              nc.sync.dma_start(out_d[:, cut:], out_sb[:, cut:])

    nc.compile()
    return nc


def _get_compiled():
    global _COMPILED
    if _COMPILED is None:
        _COMPILED = _build_program()
    return _COMPILED


# ---------------------------------------------------------------------------
# Host-side packing / scatter
# ---------------------------------------------------------------------------

def make_inputs(span_rep, Wl, bl, Wr, br, Wout, bout):
    span_rep = np.ascontiguousarray(np.asarray(span_rep, np.float32))
    Wl = np.asarray(Wl, np.float32)
    Wr = np.asarray(Wr, np.float32)
    Wout = np.asarray(Wout, np.float32)
    blbr = (np.asarray(bl, np.float32) + np.asarray(br, np.float32))

    def pack_ht(M, width):      # [512, width] f32 -> [128, HT*width] f16
        o = np.empty((128, HT * width), np.float16)
        for hi in range(HT):
            o[:, hi * width:(hi + 1) * width] = M[hi * 128:(hi + 1) * 128]
        return o

    def pack_w(Wd, Wc):         # [128, 2*HT*HID] f16
        o = np.empty((128, 2 * HT * HID), np.float16)
        for t in range(HT):
            for kind, M in ((0, Wd), (1, Wc)):
                for hi in range(HT):
                    c0 = t * 1024 + kind * HID + hi * 128
                    o[:, c0:c0 + 128] = \
                        M[hi * 128:(hi + 1) * 128, t * 128:(t + 1) * 128]
        return o

    wp_n = pack_w(Wr, Wl)       # normal cores: dense=R(Wr), col=L(Wl)
    wp_t = pack_w(Wl, Wr)       # transposed:   dense=L(Wl), col=R(Wr)
    misc = np.zeros((128, HID + HT * OUT), np.float16)
    misc[0, 0:HID] = blbr.astype(np.float16)
    for t in range(HT):
        misc[:, HID + OUT * t:HID + OUT * (t + 1)] = Wout[t * 128:(t + 1) * 128]

    in_maps = []
    for core in range(NCORES):
        # pair-major packing: [pair][hin-block][cols]
        spd = np.zeros((128, HT * 4 * DW), np.float16)
        spc = np.zeros((128, HT * 4 * DC), np.float16)
        for p, (sa, sb) in enumerate(PAIRS):
            ppos = PAIR_POS[p]
            dblk = np.zeros((HID, DW), np.float32)
            cblk = np.zeros((HID, DC), np.float32)
            for si, s in enumerate((sa, sb)):
                j = jmap(core, s)
                if j is None:
                    continue
                if core < 4:    # normal: a=i (count j), w=k (count 64-j)
                    dn = span_rep[j, j + 1:65].T       # [512, 64-j]
                    cn = span_rep[0:j, j].T            # [512, j]
                else:           # transposed: a=k, w=i
                    dn = span_rep[0:j, j].T            # [512, j]
                    cn = span_rep[j, j + 1:65].T       # [512, 64-j]
                dd = 0 if si == 0 else SLOTS[sa][1]
                cc = 0 if si == 0 else 2 * SLOTS[sa][0]
                dblk[:, dd:dd + dn.shape[1]] = dn
                cblk[:, cc:cc + 2 * cn.shape[1]:2] = cn
                cblk[:, cc + 1:cc + 2 * cn.shape[1]:2] = cn
            for hi in range(HT):
                spd[:, (ppos * HT + hi) * DW:(ppos * HT + hi + 1) * DW] = \
                    dblk[hi * 128:(hi + 1) * 128]
                spc[:, (ppos * HT + hi) * DC:(ppos * HT + hi + 1) * DC] = \
                    cblk[hi * 128:(hi + 1) * 128]
        in_maps.append({
            "wp": wp_n if core < 4 else wp_t,
            "spd": spd,
            "spc": spc,
            "misc": misc,
        })
    return in_maps


def scatter_outputs(core_outs, bout):
    bout = np.asarray(bout, np.float32)
    full = np.zeros((N1, N1, N1, OUT), np.float32)
    for core in range(NCORES):
        oc = np.asarray(core_outs[core])
        for s in range(8):
            j = jmap(core, s)
            if j is None:
                continue
            A, W = SLOTS[s]
            blk = oc[:, _s_off[s]:_s_off[s] + A * W].reshape(OUT, A, W)
            if core < 4:
                full[0:j, j, j + 1:65, :] = \
                    blk[:, 0:j, 0:64 - j].transpose(1, 2, 0) + bout
            else:
                full[0:j, j, j + 1:65, :] = \
                    blk[:, 0:64 - j, 0:j].transpose(2, 1, 0) + bout
    return full


def kernel(span_rep, Wl, bl, Wr, br, Wout, bout):
    from concourse.bass_utils import run_bass_kernel_spmd

    nc = _get_compiled()
    in_maps = make_inputs(span_rep, Wl, bl, Wr, br, Wout, bout)
    res = run_bass_kernel_spmd(nc, in_maps, core_ids=list(range(NCORES)))
    core_outs = [res.results[c]["out"] for c in range(NCORES)]
    return scatter_outputs(core_outs, bout)


if __name__ == "__main__":
    rng = np.random.default_rng(0)
    s = 1.0 / np.sqrt(HID)
    inputs = dict(
        span_rep=rng.standard_normal((N1, N1, HID)).astype(np.float32),
        Wl=(rng.standard_normal((HID, HID)) * s).astype(np.float32),
        bl=np.zeros(HID, np.float32),
        Wr=(rng.standard_normal((HID, HID)) * s).astype(np.float32),
        br=np.zeros(HID, np.float32),
        Wout=(rng.standard_normal((HID, OUT)) * s).astype(np.float32),
        bout=np.zeros(OUT, np.float32),
    )
    out = kernel(**inputs)
    print("out", out.shape, out.dtype, np.abs(out).max())

    # host-side check against a numpy reference
    L = inputs["span_rep"] @ inputs["Wl"] + inputs["bl"]
    R = inputs["span_rep"] @ inputs["Wr"] + inputs["br"]
    idx = np.arange(N1)
    valid = (idx[:, None, None] < idx[None, :, None]) & \
            (idx[None, :, None] < idx[None, None, :])
    Hf = np.tanh(L[:, :, None, :] + R[None, :, :, :])
    exp = (Hf @ inputs["Wout"] + inputs["bout"]) * valid[..., None]
    rel = np.abs(out - exp).max() / np.abs(exp).max()
    print("rel err vs numpy reference:", rel)



# revision 2
# speedup vs baseline: 1.0617x; 1.0617x over previous
"""Trainium2 Bass kernel for nn_BTGRule — per-core exact-shape design (v3).

Reference computation:
    L = span_rep @ Wl + bl            # [65, 65, 512]
    R = span_rep @ Wr + br            # [65, 65, 512]
    H = tanh(L[i, j] + R[j, k])       # over valid triples i < j < k
    scores[i, j, k] = H @ Wout + bout # [65, 65, 65, 2]

v3 drops the SPMD single-program constraint: each core gets its own
compiled program with EXACT block shapes (j-blocks (a, w), a = min(j,
64-j), w = 64-a, transposed for j > 32), so the ~20% slot padding of the
v2 design disappears.  Cores are assigned 8 j's each (63 total) with LPT
balancing of the H-column count.

Per-core pipeline (per rep):
  PE : span projections per hout-tile (Wl/Wr groups, chunked psum), then
       score matmuls of the PREVIOUS rep (software pipelining).
  DVE: proj PSUM->SBUF f16 copies, fused broadcast-add L+R per (h-tile,
       block) via tensor_tensor with duplicated-pair APs (2x mode), and
       2/3 of the score copies.
  ACT: tanh per h-tile (4 big instrs, FD ~5500) with the (bl+br) bias
       applied FREE via the activation bias operand; 1/3 of score copies.
  H layout is h-tile-major so each tanh covers one h-tile => per-
       partition bias is exact.
  Scores are f16 (well within tolerance), host adds bout + upcasts.
"""

import numpy as np

N1 = 65
HID = 512
HT = 4            # 128-row h tiles
OUT = 2
NCORES = 8


# ---------------------------------------------------------------------------
# Block assignment: j in [1, 63], area j*(64-j); LPT onto 8 cores (<=8 each)
# ---------------------------------------------------------------------------

# Precomputed swap-optimized partition of j=1..63: per-core padded H-column
# counts are 5522-5526 (ideal 5460 with odd-w padding included).
CORE_JS = [
    [3, 8, 18, 25, 30, 31, 38],
    [1, 2, 19, 28, 35, 40, 49, 50],
    [4, 12, 20, 33, 36, 51, 53, 56],
    [13, 17, 21, 39, 42, 45, 61, 62],
    [5, 26, 32, 41, 44, 46, 59, 60],
    [6, 15, 16, 27, 34, 43, 52, 63],
    [11, 14, 29, 37, 48, 54, 55, 58],
    [7, 9, 10, 22, 23, 24, 47, 57],
]


class Block:
    def __init__(self, j):
        self.j = j
        self.trans = j > 32
        self.a = j if not self.trans else 64 - j
        self.w = 64 - self.a
        self.wpad = self.w + (self.w % 2)          # even dense width
        self.area = self.a * self.wpad
        # offsets filled by CoreLayout
        self.boff = None          # H col offset within an h-tile section
        self.dsec = None          # 'l' or 'r': dense spans weight group
        self.doff = None          # dense offset within its D section
        self.coff = None          # col offset (units of spans) in C section


class CoreLayout:
    """Span/H/score layout for one core's block list."""

    def __init__(self, js):
        self.blocks = [Block(j) for j in js]
        # H layout (within one h-tile section), block order as given
        off = 0
        for b in self.blocks:
            b.boff = off
            off += b.area
        self.S = off                                   # h-tile section cols
        # span sections: [WlD | WlC | WrD | WrC]
        # normal: col side=L(Wl dup), dense=R(Wr); transposed: col=R, dense=L
        wld = wlc = wrd = wrc = 0
        for b in self.blocks:
            if b.trans:
                b.dsec = 'l'
                b.doff = wld
                wld += b.wpad
                b.coff = wrc
                wrc += b.a
            else:
                b.dsec = 'r'
                b.doff = wrd
                wrd += b.wpad
                b.coff = wlc
                wlc += b.a
        self.WlD, self.WlC, self.WrD, self.WrC = wld, 2 * wlc, wrd, 2 * wrc
        self.o_wld = 0
        self.o_wlc = self.o_wld + self.WlD
        self.o_wrd = self.o_wlc + self.WlC
        self.o_wrc = self.o_wrd + self.WrD
        self.SPANCOLS = self.o_wrc + self.WrC
        assert self.SPANCOLS % 2 == 0
        # proj psum chunks (within Wl region [0, o_wrd) and Wr region)
        self.wl_cols = self.WlD + self.WlC
        self.wr_cols = self.WrD + self.WrC
        self.pchunks = []          # (offset, width, wgrp)
        for base, cols, grp in ((0, self.wl_cols, 'l'),
                                (self.o_wrd, self.wr_cols, 'r')):
            n = -(-cols // 512)
            step = -(-cols // (2 * n)) * 2
            c = 0
            while c < cols:
                w = min(step, cols - c)
                self.pchunks.append((base + c, w, grp))
                c += w
        # score chunks over [0, S)
        n = -(-self.S // 512)
        step = -(-self.S // (2 * n)) * 2
        self.schunks = []
        c = 0
        while c < self.S:
            w = min(step, self.S - c)
            self.schunks.append((c, w))
            c += w

    def dense_off(self, b):        # abs span-col offset of block's dense run
        return (self.o_wld if b.dsec == 'l' else self.o_wrd) + b.doff

    def col_off(self, b):          # abs span-col offset of block's dup cols
        return (self.o_wlc if b.dsec == 'r' else self.o_wrc) + 2 * b.coff


LAYOUTS = [CoreLayout(js) for js in CORE_JS]


# ---------------------------------------------------------------------------
# Program builder (per core)
# ---------------------------------------------------------------------------

_COMPILED = {}

# schedule knobs (tuned via TimelineSim)
CFG = {
    "ps_pr_bufs": 2,     # proj psum pool buffers
    "ps_sc_bufs": 6,     # score psum pool buffers
    "sc_mode": ["post", "post", "end", "end",
                "end", "end", "end", "end"],   # per-core schedule shape
    "act_step": 3,       # every act_step-th score copy goes to ACT
    "act_pc": [0, 1, 1, 1, 1, 1, 1, 1],   # per-core: proj copies on ACT
    "sc_pair": 1,        # score chunks per psum tile
}


def _cfg(key, core):
    v = CFG[key]
    return v[core] if isinstance(v, (list, tuple)) else v


def _build_program(core, reps=1, unroll=False, inner=1):
    import contextlib

    import concourse.bacc as bacc
    import concourse.mybir as mybir
    import concourse.tile as tile

    lay = LAYOUTS[core]
    f32 = mybir.dt.float32
    f16 = mybir.dt.float16
    tanh = mybir.ActivationFunctionType.Tanh
    ident = mybir.ActivationFunctionType.Identity
    add = mybir.AluOpType.add

    SC, S = lay.SPANCOLS, lay.S

    nc = bacc.Bacc("TRN2", target_bir_lowering=False, debug=False,
                   num_devices=1)

    wp_d = nc.declare_dram_parameter("wp", [128, 2 * HT * HID], f16,
                                     isOutput=False)
    sp_d = nc.declare_dram_parameter("sp", [128, HT * SC], f16,
                                     isOutput=False)
    wout_d = nc.declare_dram_parameter("wout", [128, HT * OUT], f16,
                                       isOutput=False)
    blbr_d = nc.declare_dram_parameter("blbr", [128, HT], f32,
                                       isOutput=False)
    out_d = nc.declare_dram_parameter("out", [OUT, S], f16, isOutput=True)

    with tile.TileContext(nc) as tc:
        with (
            tc.tile_pool(name="stream", bufs=2) as spool,
            tc.tile_pool(name="hbuf", bufs=2) as hpool,
            tc.tile_pool(name="obuf", bufs=2) as opool,
            tc.tile_pool(name="ps_pr", bufs=CFG["ps_pr_bufs"],
                         space="PSUM") as ps_pr,
            tc.tile_pool(name="ps_sc", bufs=CFG["ps_sc_bufs"],
                         space="PSUM") as ps_sc,
            tc.For_i(0, reps // inner, 1,
                     hint_engines=(mybir.EngineType.PE,
                                   mybir.EngineType.DVE,
                                   mybir.EngineType.Activation,
                                   mybir.EngineType.SP))
            if reps > inner and not unroll else contextlib.nullcontext(),
        ):
            prev = [None]          # (H_t, wout_t) of the previous rep
            NS = len(lay.schunks)

            # score groups: schunks paired (sc_pair) into one psum tile so a
            # single evacuation copy covers the pair
            PAIR = CFG.get("sc_pair", 1)
            SGROUPS = [lay.schunks[g:g + PAIR]
                       for g in range(0, NS, PAIR)]
            NG = len(SGROUPS)
            SLOT_GROUPS = [list(range((NG * t) // HT, (NG * (t + 1)) // HT))
                           for t in range(HT)]
            ACT_GROUPS = set(range(CFG["act_step"] - 1, NG, CFG["act_step"]))

            def emit_score_mms(st, gis):
                H_t, wout_t = st
                out = []
                for gi in gis:
                    chunks = SGROUPS[gi]
                    g0 = chunks[0][0]
                    gw = sum(cw for _, cw in chunks)
                    psc = ps_sc.tile([OUT, gw], f32, tag="pssc")
                    for (c0, cw) in chunks:
                        for t in range(HT):
                            nc.tensor.matmul(
                                psc[:, c0 - g0:c0 - g0 + cw],
                                wout_t[:, OUT * t:OUT * (t + 1)],
                                H_t[:, t * S + c0:t * S + c0 + cw],
                                start=(t == 0), stop=(t == HT - 1))
                    out.append((gi, g0, gw, psc))
                return out

            def emit_score_copies(out_sb, pend):
                for gi, g0, gw, psc in pend:
                    dst = out_sb[:, g0:g0 + gw]
                    if gi in ACT_GROUPS:
                        nc.scalar.activation(dst, psc[:], ident)
                    else:
                        nc.vector.tensor_copy(dst, psc[:])
                if pend:
                    g0 = pend[0][1]
                    g1, gw1 = pend[-1][1], pend[-1][2]
                    nc.sync.dma_start(out_d[:, g0:g1 + gw1],
                                      out_sb[:, g0:g1 + gw1])

            for _rep in range(reps if unroll else inner):
                sp_t = spool.tile([128, HT * SC], f16, tag="sp")
                wp_t = spool.tile([128, 2 * HT * HID], f16, tag="wp")
                wout_t = spool.tile([128, HT * OUT], f16, tag="wout")
                blbr_t = spool.tile([128, HT], f32, tag="blbr")
                nc.sync.dma_start(sp_t[:, 0:2 * SC], sp_d[:, 0:2 * SC])
                nc.scalar.dma_start(wp_t[:, 0:2048], wp_d[:, 0:2048])
                nc.sync.dma_start(sp_t[:, 2 * SC:], sp_d[:, 2 * SC:])
                nc.scalar.dma_start(wp_t[:, 2048:], wp_d[:, 2048:])
                nc.scalar.dma_start(wout_t[:], wout_d[:])
                nc.scalar.dma_start(blbr_t[:], blbr_d[:])

                # weight block: wgrp l/r, hout t, hin hi
                def wblk(grp, t, hi):
                    kind = 0 if grp == 'l' else 1
                    c0 = t * 1024 + kind * HID + hi * 128
                    return wp_t[:, c0:c0 + 128]

                proj_sb = spool.tile([128, HT * SC], f16, tag="proj")
                H_t = hpool.tile([128, HT * S], f16, tag="H")
                if prev[0] is not None:
                    out_sb = opool.tile([OUT, S], f16, tag="osb")
                else:
                    out_sb = None
                pend = []

                pc_cnt = [0]
                for t in range(HT):
                    if _cfg("sc_mode", core) == "pre" and pend:
                        emit_score_copies(out_sb, pend)
                        pend = []
                    # projections for hout-tile t (PE) + copies (DVE/ACT)
                    for (c0, cw, grp) in lay.pchunks:
                        ps = ps_pr.tile([128, cw], f32, tag="pspr")
                        for hi in range(HT):
                            nc.tensor.matmul(
                                ps[:], wblk(grp, t, hi),
                                sp_t[:, hi * SC + c0:hi * SC + c0 + cw],
                                start=(hi == 0), stop=(hi == HT - 1))
                        pdst = proj_sb[:, t * SC + c0:t * SC + c0 + cw]
                        if pc_cnt[0] < _cfg("act_pc", core):
                            nc.scalar.activation(pdst, ps[:], ident)
                        else:
                            nc.vector.tensor_copy(pdst, ps[:])
                        pc_cnt[0] += 1
                    # broadcast adds for all blocks, h-tile t (DVE)
                    for b in lay.blocks:
                        h0 = t * S + b.boff
                        out_v = (H_t[:, h0:h0 + b.area]
                                 .rearrange("p (a w2 two) -> p a w2 two",
                                            a=b.a, two=2))
                        d0 = t * SC + lay.dense_off(b)
                        in0 = (proj_sb[:, d0:d0 + b.wpad]
                               .rearrange("p (w2 two) -> p w2 two", two=2)
                               .unsqueeze(1)
                               .broadcast_to([128, b.a, b.wpad // 2, 2]))
                        c0 = t * SC + lay.col_off(b)
                        in1 = (proj_sb[:, c0:c0 + 2 * b.a]
                               .rearrange("p (a two) -> p a two", two=2)
                               .unsqueeze(2)
                               .broadcast_to([128, b.a, b.wpad // 2, 2]))
                        nc.vector.tensor_tensor(out_v, in0, in1, op=add)
                    # tanh with free (bl+br) bias for this h-tile (ACT)
                    sec = H_t[:, t * S:(t + 1) * S]
                    nc.scalar.activation(sec, sec, tanh,
                                         bias=blbr_t[:, t:t + 1])
                    # previous rep's scores, interleaved at h-tile cadence
                    if prev[0] is not None and _cfg("sc_mode", core) != "end":
                        pend += emit_score_mms(prev[0], SLOT_GROUPS[t])
                        if _cfg("sc_mode", core) == "post":
                            emit_score_copies(out_sb, pend)
                            pend = []
                if prev[0] is not None:
                    if _cfg("sc_mode", core) == "end":
                        pend = emit_score_mms(prev[0], list(range(NG)))
                    if pend:
                        emit_score_copies(out_sb, pend)
                prev[0] = (H_t, wout_t)

            out_sb = opool.tile([OUT, S], f16, tag="osb")
            pend = emit_score_mms(prev[0], list(range(NG)))
            emit_score_copies(out_sb, pend)

    nc.compile()
    return nc


def _get_compiled(core):
    if core not in _COMPILED:
        _COMPILED[core] = _build_program(core)
    return _COMPILED[core]


# ---------------------------------------------------------------------------
# Host-side packing / scatter
# ---------------------------------------------------------------------------

def make_inputs(span_rep, Wl, bl, Wr, br, Wout, bout):
    span_rep = np.ascontiguousarray(np.asarray(span_rep, np.float32))
    Wl = np.asarray(Wl, np.float32)
    Wr = np.asarray(Wr, np.float32)
    Wout = np.asarray(Wout, np.float32)
    blbr = np.asarray(bl, np.float32) + np.asarray(br, np.float32)

    # wp: per (hout t, kind, hin hi) 128x128 blocks; kind 0 = Wl, 1 = Wr
    wp = np.empty((128, 2 * HT * HID), np.float16)
    for t in range(HT):
        for kind, M in ((0, Wl), (1, Wr)):
            for hi in range(HT):
                c0 = t * 1024 + kind * HID + hi * 128
                wp[:, c0:c0 + 128] = \
                    M[hi * 128:(hi + 1) * 128, t * 128:(t + 1) * 128]
    wout_p = np.empty((128, HT * OUT), np.float16)
    for t in range(HT):
        wout_p[:, OUT * t:OUT * (t + 1)] = Wout[t * 128:(t + 1) * 128]
    blbr_p = np.empty((128, HT), np.float32)
    for t in range(HT):
        blbr_p[:, t] = blbr[t * 128:(t + 1) * 128]

    in_maps = []
    for core in range(NCORES):
        lay = LAYOUTS[core]
        spc = np.zeros((HID, lay.SPANCOLS), np.float32)
        for b in lay.blocks:
            j = b.j
            left = span_rep[0:j, j].T          # [512, j]  (L spans)
            right = span_rep[j, j + 1:65].T    # [512, 64-j]  (R spans)
            dense, col = (left, right) if b.trans else (right, left)
            d0 = lay.dense_off(b)
            spc[:, d0:d0 + b.w] = dense
            c0 = lay.col_off(b)
            spc[:, c0:c0 + 2 * b.a:2] = col
            spc[:, c0 + 1:c0 + 2 * b.a:2] = col
        sp = np.empty((128, HT * lay.SPANCOLS), np.float16)
        for hi in range(HT):
            sp[:, hi * lay.SPANCOLS:(hi + 1) * lay.SPANCOLS] = \
                spc[hi * 128:(hi + 1) * 128]
        in_maps.append({"wp": wp, "sp": sp, "wout": wout_p, "blbr": blbr_p})
    return in_maps


def scatter_outputs(core_outs, bout):
    bout = np.asarray(bout, np.float32)
    full = np.zeros((N1, N1, N1, OUT), np.float32)
    for core in range(NCORES):
        lay = LAYOUTS[core]
        oc = np.asarray(core_outs[core], np.float32)
        for b in lay.blocks:
            j = b.j
            blk = oc[:, b.boff:b.boff + b.area].reshape(OUT, b.a, b.wpad)
            if b.trans:
                full[0:j, j, j + 1:65, :] = \
                    blk[:, 0:64 - j, 0:j].transpose(2, 1, 0) + bout
            else:
                full[0:j, j, j + 1:65, :] = \
                    blk[:, 0:j, 0:64 - j].transpose(1, 2, 0) + bout
    return full


def kernel(span_rep, Wl, bl, Wr, br, Wout, bout):
    from concourse.bass_utils import run_bass_kernel_spmd

    in_maps = make_inputs(span_rep, Wl, bl, Wr, br, Wout, bout)
    core_outs = []
    for core in range(NCORES):
        nc = _get_compiled(core)
        res = run_bass_kernel_spmd(nc, [in_maps[core]], core_ids=[0])
        core_outs.append(res.results[0]["out"])
    return scatter_outputs(core_outs, bout)


if __name__ == "__main__":
    for c, js in enumerate(CORE_JS):
        lay = LAYOUTS[c]
        print(f"core {c}: js={sorted(js)} S={lay.S} SPANCOLS={lay.SPANCOLS} "
              f"pchunks={len(lay.pchunks)} schunks={len(lay.schunks)}")


# revision 3
# speedup vs baseline: 1.1174x; 1.0525x over previous
"""Trainium2 Bass kernel for nn_BTGRule — per-core exact-shape design (v3).

Reference computation:
    L = span_rep @ Wl + bl            # [65, 65, 512]
    R = span_rep @ Wr + br            # [65, 65, 512]
    H = tanh(L[i, j] + R[j, k])       # over valid triples i < j < k
    scores[i, j, k] = H @ Wout + bout # [65, 65, 65, 2]

v3 drops the SPMD single-program constraint: each core gets its own
compiled program with EXACT block shapes (j-blocks (a, w), a = min(j,
64-j), w = 64-a, transposed for j > 32), so the ~20% slot padding of the
v2 design disappears.  Cores are assigned 8 j's each (63 total) with LPT
balancing of the H-column count.

Per-core pipeline (per rep):
  PE : span projections per hout-tile (Wl/Wr groups, chunked psum), then
       score matmuls of the PREVIOUS rep (software pipelining).
  DVE: proj PSUM->SBUF f16 copies, fused broadcast-add L+R per (h-tile,
       block) via tensor_tensor with duplicated-pair APs (2x mode), and
       2/3 of the score copies.
  ACT: tanh per h-tile (4 big instrs, FD ~5500) with the (bl+br) bias
       applied FREE via the activation bias operand; 1/3 of score copies.
  H layout is h-tile-major so each tanh covers one h-tile => per-
       partition bias is exact.
  Scores are f16 (well within tolerance), host adds bout + upcasts.
"""

import numpy as np

N1 = 65
HID = 512
HT = 4            # 128-row h tiles
OUT = 2
NCORES = 8


# ---------------------------------------------------------------------------
# Block assignment: j in [1, 63], area j*(64-j); LPT onto 8 cores (<=8 each)
# ---------------------------------------------------------------------------

# Precomputed swap-optimized partition of j=1..63: per-core padded H-column
# counts are 5522-5526 (ideal 5460 with odd-w padding included).
CORE_JS = [
    [3, 8, 18, 25, 30, 31, 38],
    [1, 2, 19, 28, 35, 40, 49, 50],
    [4, 12, 20, 33, 36, 51, 53, 56],
    [13, 17, 21, 39, 42, 45, 61, 62],
    [5, 26, 32, 41, 44, 46, 59, 60],
    [6, 15, 16, 27, 34, 43, 52, 63],
    [11, 14, 29, 37, 48, 54, 55, 58],
    [7, 9, 10, 22, 23, 24, 47, 57],
]


class Block:
    def __init__(self, j):
        self.j = j
        self.trans = j > 32
        self.a = j if not self.trans else 64 - j
        self.w = 64 - self.a
        self.wpad = self.w + (self.w % 2)          # even dense width
        self.area = self.a * self.wpad
        # offsets filled by CoreLayout
        self.boff = None          # H col offset within an h-tile section
        self.dsec = None          # 'l' or 'r': dense spans weight group
        self.doff = None          # dense offset within its D section
        self.coff = None          # col offset (units of spans) in C section


class CoreLayout:
    """Span/H/score layout for one core's block list."""

    def __init__(self, js):
        self.blocks = [Block(j) for j in js]
        # H layout (within one h-tile section), block order as given
        off = 0
        for b in self.blocks:
            b.boff = off
            off += b.area
        self.S = off                                   # h-tile section cols
        # span sections: [WlD | WlC | WrD | WrC]
        # normal: col side=L(Wl dup), dense=R(Wr); transposed: col=R, dense=L
        wld = wlc = wrd = wrc = 0
        for b in self.blocks:
            if b.trans:
                b.dsec = 'l'
                b.doff = wld
                wld += b.wpad
                b.coff = wrc
                wrc += b.a
            else:
                b.dsec = 'r'
                b.doff = wrd
                wrd += b.wpad
                b.coff = wlc
                wlc += b.a
        self.WlD, self.WlC, self.WrD, self.WrC = wld, 2 * wlc, wrd, 2 * wrc
        self.o_wld = 0
        self.o_wlc = self.o_wld + self.WlD
        self.o_wrd = self.o_wlc + self.WlC
        self.o_wrc = self.o_wrd + self.WrD
        self.SPANCOLS = self.o_wrc + self.WrC
        assert self.SPANCOLS % 2 == 0
        # proj psum chunks (within Wl region [0, o_wrd) and Wr region)
        self.wl_cols = self.WlD + self.WlC
        self.wr_cols = self.WrD + self.WrC
        self.pchunks = []          # (offset, width, wgrp)
        for base, cols, grp in ((0, self.wl_cols, 'l'),
                                (self.o_wrd, self.wr_cols, 'r')):
            n = -(-cols // 512)
            step = -(-cols // (2 * n)) * 2
            c = 0
            while c < cols:
                w = min(step, cols - c)
                self.pchunks.append((base + c, w, grp))
                c += w
        # score chunks over [0, S)
        n = -(-self.S // 512)
        step = -(-self.S // (2 * n)) * 2
        self.schunks = []
        c = 0
        while c < self.S:
            w = min(step, self.S - c)
            self.schunks.append((c, w))
            c += w

    def dense_off(self, b):        # abs span-col offset of block's dense run
        return (self.o_wld if b.dsec == 'l' else self.o_wrd) + b.doff

    def col_off(self, b):          # abs span-col offset of block's dup cols
        return (self.o_wlc if b.dsec == 'r' else self.o_wrc) + 2 * b.coff


LAYOUTS = [CoreLayout(js) for js in CORE_JS]


# ---------------------------------------------------------------------------
# Program builder (per core)
# ---------------------------------------------------------------------------

_COMPILED = {}

# schedule knobs (tuned via TimelineSim)
CFG = {
    "ps_pr_bufs": 2,     # proj psum pool buffers
    "ps_sc_bufs": 6,     # score psum pool buffers
    "sc_mode": "end",    # prev-rep scores emitted at body end
    "act_step": 3,       # every act_step-th score copy goes to ACT
    "act_pc": 0,         # number of proj copies (of 8) done on ACT
    "sc_pair": 1,        # score chunks per psum tile
    "tanh_merge": 1,     # h-tile sections per tanh instr (1: bias via ACT;
                         # >1: bias added in projections via ones-matmul)
    "wp_q": "gpsimd",    # wp (weights) DMA issued from the idle Pool queue
}


def _cfg(key, core):
    v = CFG[key]
    return v[core] if isinstance(v, (list, tuple)) else v


def _build_program(core, reps=1, unroll=False, inner=1):
    import contextlib

    import concourse.bacc as bacc
    import concourse.mybir as mybir
    import concourse.tile as tile

    lay = LAYOUTS[core]
    f32 = mybir.dt.float32
    f16 = mybir.dt.float16
    tanh = mybir.ActivationFunctionType.Tanh
    ident = mybir.ActivationFunctionType.Identity
    add = mybir.AluOpType.add

    SC, S = lay.SPANCOLS, lay.S

    nc = bacc.Bacc("TRN2", target_bir_lowering=False, debug=False,
                   num_devices=1)

    wp_d = nc.declare_dram_parameter("wp", [128, 2 * HT * HID], f16,
                                     isOutput=False)
    sp_d = nc.declare_dram_parameter("sp", [128, HT * SC], f16,
                                     isOutput=False)
    wout_d = nc.declare_dram_parameter("wout", [128, HT * OUT], f16,
                                       isOutput=False)
    blbr_d = nc.declare_dram_parameter("blbr", [128, HT], f32,
                                       isOutput=False)
    MERGE = _cfg("tanh_merge", core)
    if MERGE > 1:
        blbr16_d = nc.declare_dram_parameter("blbr16", [1, HID], f16,
                                             isOutput=False)
    out_d = nc.declare_dram_parameter("out", [OUT, S], f16, isOutput=True)

    with tile.TileContext(nc) as tc:
        with (
            tc.tile_pool(name="stream", bufs=2) as spool,
            tc.tile_pool(name="hbuf", bufs=2) as hpool,
            tc.tile_pool(name="obuf", bufs=2) as opool,
            tc.tile_pool(name="ps_pr", bufs=CFG["ps_pr_bufs"],
                         space="PSUM") as ps_pr,
            tc.tile_pool(name="ps_sc", bufs=CFG["ps_sc_bufs"],
                         space="PSUM") as ps_sc,
            tc.For_i(0, reps // inner, 1,
                     hint_engines=(mybir.EngineType.PE,
                                   mybir.EngineType.DVE,
                                   mybir.EngineType.Activation,
                                   mybir.EngineType.SP))
            if reps > inner and not unroll else contextlib.nullcontext(),
        ):
            prev = [None]          # (H_t, wout_t) of the previous rep
            NS = len(lay.schunks)

            # score groups: schunks paired (sc_pair) into one psum tile so a
            # single evacuation copy covers the pair
            PAIR = CFG.get("sc_pair", 1)
            SGROUPS = [lay.schunks[g:g + PAIR]
                       for g in range(0, NS, PAIR)]
            NG = len(SGROUPS)
            SLOT_GROUPS = [list(range((NG * t) // HT, (NG * (t + 1)) // HT))
                           for t in range(HT)]
            ACT_GROUPS = set(range(CFG["act_step"] - 1, NG, CFG["act_step"]))

            def emit_score_mms(st, gis):
                H_t, wout_t = st
                out = []
                for gi in gis:
                    chunks = SGROUPS[gi]
                    g0 = chunks[0][0]
                    gw = sum(cw for _, cw in chunks)
                    psc = ps_sc.tile([OUT, gw], f32, tag="pssc")
                    for (c0, cw) in chunks:
                        for t in range(HT):
                            nc.tensor.matmul(
                                psc[:, c0 - g0:c0 - g0 + cw],
                                wout_t[:, OUT * t:OUT * (t + 1)],
                                H_t[:, t * S + c0:t * S + c0 + cw],
                                start=(t == 0), stop=(t == HT - 1))
                    out.append((gi, g0, gw, psc))
                return out

            def emit_score_copies(out_sb, pend):
                for gi, g0, gw, psc in pend:
                    dst = out_sb[:, g0:g0 + gw]
                    if gi in ACT_GROUPS:
                        nc.scalar.activation(dst, psc[:], ident)
                    else:
                        nc.vector.tensor_copy(dst, psc[:])
                if pend:
                    g0 = pend[0][1]
                    g1, gw1 = pend[-1][1], pend[-1][2]
                    nc.sync.dma_start(out_d[:, g0:g1 + gw1],
                                      out_sb[:, g0:g1 + gw1])

            for _rep in range(reps if unroll else inner):
                sp_t = spool.tile([128, HT * SC], f16, tag="sp")
                wp_t = spool.tile([128, 2 * HT * HID], f16, tag="wp")
                wout_t = spool.tile([128, HT * OUT], f16, tag="wout")
                blbr_t = spool.tile([128, HT], f32, tag="blbr")
                wq = (nc.gpsimd if _cfg("wp_q", core) == "gpsimd"
                      else nc.scalar)
                nc.sync.dma_start(sp_t[:, 0:2 * SC], sp_d[:, 0:2 * SC])
                wq.dma_start(wp_t[:, 0:2048], wp_d[:, 0:2048])
                nc.sync.dma_start(sp_t[:, 2 * SC:], sp_d[:, 2 * SC:])
                wq.dma_start(wp_t[:, 2048:], wp_d[:, 2048:])
                nc.scalar.dma_start(wout_t[:], wout_d[:])
                nc.scalar.dma_start(blbr_t[:], blbr_d[:])
                if MERGE > 1:
                    blbr16_t = spool.tile([1, HID], f16, tag="blbr16")
                    nc.scalar.dma_start(blbr16_t[:], blbr16_d[:])
                    ones_t = spool.tile([1, 512], f16, tag="ones")
                    nc.vector.memset(ones_t[:], 1.0)

                # weight block: wgrp l/r, hout t, hin hi
                def wblk(grp, t, hi):
                    kind = 0 if grp == 'l' else 1
                    c0 = t * 1024 + kind * HID + hi * 128
                    return wp_t[:, c0:c0 + 128]

                proj_sb = spool.tile([128, HT * SC], f16, tag="proj")
                H_t = hpool.tile([128, HT * S], f16, tag="H")
                if prev[0] is not None:
                    out_sb = opool.tile([OUT, S], f16, tag="osb")
                else:
                    out_sb = None
                pend = []

                pc_cnt = [0]
                for t in range(HT):
                    if _cfg("sc_mode", core) == "pre" and pend:
                        emit_score_copies(out_sb, pend)
                        pend = []
                    # projections for hout-tile t (PE) + copies (DVE/ACT)
                    for (c0, cw, grp) in lay.pchunks:
                        ps = ps_pr.tile([128, cw], f32, tag="pspr")
                        if MERGE > 1:
                            # split chunk into D / C parts; (bl+br) is folded
                            # into the C (duplicated-column) operands here so
                            # merged tanh instrs need no per-partition bias
                            crange = ((lay.o_wlc, lay.o_wrd) if grp == 'l'
                                      else (lay.o_wrc, lay.SPANCOLS))
                            cv0 = max(c0, crange[0])
                            cv1 = min(c0 + cw, crange[1])
                            parts = []
                            if cv0 > c0:
                                parts.append((c0, cv0, False))
                            if cv1 > cv0:
                                parts.append((cv0, cv1, True))
                            for (p0, p1, isc) in parts:
                                for hi in range(HT):
                                    nc.tensor.matmul(
                                        ps[:, p0 - c0:p1 - c0],
                                        wblk(grp, t, hi),
                                        sp_t[:, hi * SC + p0:hi * SC + p1],
                                        start=(hi == 0),
                                        stop=(hi == HT - 1) and not isc)
                                if isc:
                                    nc.tensor.matmul(
                                        ps[:, p0 - c0:p1 - c0],
                                        blbr16_t[0:1,
                                                 t * 128:(t + 1) * 128],
                                        ones_t[0:1, 0:p1 - p0],
                                        start=False, stop=True)
                        else:
                            for hi in range(HT):
                                nc.tensor.matmul(
                                    ps[:], wblk(grp, t, hi),
                                    sp_t[:, hi * SC + c0:hi * SC + c0 + cw],
                                    start=(hi == 0), stop=(hi == HT - 1))
                        pdst = proj_sb[:, t * SC + c0:t * SC + c0 + cw]
                        if pc_cnt[0] < _cfg("act_pc", core):
                            nc.scalar.activation(pdst, ps[:], ident)
                        else:
                            nc.vector.tensor_copy(pdst, ps[:])
                        pc_cnt[0] += 1
                    # broadcast adds for all blocks, h-tile t (DVE)
                    for b in lay.blocks:
                        h0 = t * S + b.boff
                        out_v = (H_t[:, h0:h0 + b.area]
                                 .rearrange("p (a w2 two) -> p a w2 two",
                                            a=b.a, two=2))
                        d0 = t * SC + lay.dense_off(b)
                        in0 = (proj_sb[:, d0:d0 + b.wpad]
                               .rearrange("p (w2 two) -> p w2 two", two=2)
                               .unsqueeze(1)
                               .broadcast_to([128, b.a, b.wpad // 2, 2]))
                        c0 = t * SC + lay.col_off(b)
                        in1 = (proj_sb[:, c0:c0 + 2 * b.a]
                               .rearrange("p (a two) -> p a two", two=2)
                               .unsqueeze(2)
                               .broadcast_to([128, b.a, b.wpad // 2, 2]))
                        nc.vector.tensor_tensor(out_v, in0, in1, op=add)
                    # tanh (bias free via ACT when unmerged, else in proj)
                    if MERGE == 1:
                        sec = H_t[:, t * S:(t + 1) * S]
                        nc.scalar.activation(sec, sec, tanh,
                                             bias=blbr_t[:, t:t + 1])
                    elif (t + 1) % MERGE == 0:
                        sec = H_t[:, (t + 1 - MERGE) * S:(t + 1) * S]
                        nc.scalar.activation(sec, sec, tanh)
                    # previous rep's scores, interleaved at h-tile cadence
                    if prev[0] is not None and _cfg("sc_mode", core) != "end":
                        pend += emit_score_mms(prev[0], SLOT_GROUPS[t])
                        if _cfg("sc_mode", core) == "post":
                            emit_score_copies(out_sb, pend)
                            pend = []
                if prev[0] is not None:
                    if _cfg("sc_mode", core) == "end":
                        pend = emit_score_mms(prev[0], list(range(NG)))
                    if pend:
                        emit_score_copies(out_sb, pend)
                prev[0] = (H_t, wout_t)

            out_sb = opool.tile([OUT, S], f16, tag="osb")
            pend = emit_score_mms(prev[0], list(range(NG)))
            emit_score_copies(out_sb, pend)

    nc.compile()
    return nc


def _get_compiled(core):
    if core not in _COMPILED:
        _COMPILED[core] = _build_program(core)
    return _COMPILED[core]


# ---------------------------------------------------------------------------
# Host-side packing / scatter
# ---------------------------------------------------------------------------

def make_inputs(span_rep, Wl, bl, Wr, br, Wout, bout):
    span_rep = np.ascontiguousarray(np.asarray(span_rep, np.float32))
    Wl = np.asarray(Wl, np.float32)
    Wr = np.asarray(Wr, np.float32)
    Wout = np.asarray(Wout, np.float32)
    blbr = np.asarray(bl, np.float32) + np.asarray(br, np.float32)

    # wp: per (hout t, kind, hin hi) 128x128 blocks; kind 0 = Wl, 1 = Wr
    wp = np.empty((128, 2 * HT * HID), np.float16)
    for t in range(HT):
        for kind, M in ((0, Wl), (1, Wr)):
            for hi in range(HT):
                c0 = t * 1024 + kind * HID + hi * 128
                wp[:, c0:c0 + 128] = \
                    M[hi * 128:(hi + 1) * 128, t * 128:(t + 1) * 128]
    wout_p = np.empty((128, HT * OUT), np.float16)
    for t in range(HT):
        wout_p[:, OUT * t:OUT * (t + 1)] = Wout[t * 128:(t + 1) * 128]
    blbr_p = np.empty((128, HT), np.float32)
    for t in range(HT):
        blbr_p[:, t] = blbr[t * 128:(t + 1) * 128]

    in_maps = []
    for core in range(NCORES):
        lay = LAYOUTS[core]
        spc = np.zeros((HID, lay.SPANCOLS), np.float32)
        for b in lay.blocks:
            j = b.j
            left = span_rep[0:j, j].T          # [512, j]  (L spans)
            right = span_rep[j, j + 1:65].T    # [512, 64-j]  (R spans)
            dense, col = (left, right) if b.trans else (right, left)
            d0 = lay.dense_off(b)
            spc[:, d0:d0 + b.w] = dense
            c0 = lay.col_off(b)
            spc[:, c0:c0 + 2 * b.a:2] = col
            spc[:, c0 + 1:c0 + 2 * b.a:2] = col
        sp = np.empty((128, HT * lay.SPANCOLS), np.float16)
        for hi in range(HT):
            sp[:, hi * lay.SPANCOLS:(hi + 1) * lay.SPANCOLS] = \
                spc[hi * 128:(hi + 1) * 128]
        in_maps.append({"wp": wp, "sp": sp, "wout": wout_p, "blbr": blbr_p,
                        "blbr16": blbr.astype(np.float16).reshape(1, HID)})
    return in_maps


def scatter_outputs(core_outs, bout):
    bout = np.asarray(bout, np.float32)
    full = np.zeros((N1, N1, N1, OUT), np.float32)
    for core in range(NCORES):
        lay = LAYOUTS[core]
        oc = np.asarray(core_outs[core], np.float32)
        for b in lay.blocks:
            j = b.j
            blk = oc[:, b.boff:b.boff + b.area].reshape(OUT, b.a, b.wpad)
            if b.trans:
                full[0:j, j, j + 1:65, :] = \
                    blk[:, 0:64 - j, 0:j].transpose(2, 1, 0) + bout
            else:
                full[0:j, j, j + 1:65, :] = \
                    blk[:, 0:j, 0:64 - j].transpose(1, 2, 0) + bout
    return full


def kernel(span_rep, Wl, bl, Wr, br, Wout, bout):
    from concourse.bass_utils import run_bass_kernel_spmd

    in_maps = make_inputs(span_rep, Wl, bl, Wr, br, Wout, bout)
    core_outs = []
    for core in range(NCORES):
        nc = _get_compiled(core)
        res = run_bass_kernel_spmd(nc, [in_maps[core]], core_ids=[0])
        core_outs.append(res.results[0]["out"])
    return scatter_outputs(core_outs, bout)


if __name__ == "__main__":
    for c, js in enumerate(CORE_JS):
        lay = LAYOUTS[c]
        print(f"core {c}: js={sorted(js)} S={lay.S} SPANCOLS={lay.SPANCOLS} "
              f"pchunks={len(lay.pchunks)} schunks={len(lay.schunks)}")
